# revision 3
# baseline (speedup 1.0000x reference)
"""Trainium2 Bass kernel for db4 wavelet high-frequency extraction.

Math: per (b,c) plane X [512,512]:
    out = A X B^T + X C^T
with A = I+E, B = I-E, C = 2E, E = S_hi @ G_hi (dwt/idwt high-band operator,
bandwidth 7).  This equals idwt2(ll, 2lh, 2hl, 2hh) of dwt2(X) (db4,
mode=symmetric) to ~2e-7 relative.

Mapping (per plane, all on the tensor engine, f32r = fp32 with 11-bit
mantissa rounding, full-rate on TRN2 when the moving operand is >=256 wide):
  stage 1: weights = X blocks (128x128), stream A^T band windows (N=256)
           -> PSUM accumulates T1^T = X^T A^T (banded windows cover [0,512))
           plus transpose-mode matmuls -> X^T
  stage 2: weights = T1^T / X^T blocks, stream B^T / C^T band windows
           -> out = T1 B^T + X C^T  in natural orientation.

Sharding: 96 (b,c) planes, 12 per core, pure data parallel on 8 cores.
"""
import numpy as np

# ---------------------------------------------------------------- constants
_DEC_LO = np.array([-0.010597401784997278, 0.032883011666982945,
                    0.030841381835986965, -0.18703481171888114,
                    -0.02798376941698385, 0.6308807679295904,
                    0.7148465705525415, 0.23037781330885523], dtype=np.float64)
_F = 8
_SIGNS = np.array([(-1.0) ** (k + 1) for k in range(_F)])
_DEC_HI = _SIGNS * _DEC_LO[::-1]
_REC_LO = _DEC_LO[::-1].copy()
_REC_HI = _DEC_HI[::-1].copy()

N = 512
M = (N + _F - 1) // 2
B_TOT, C_TOT, PLANES_PER_CORE, N_CORES = 32, 3, 12, 8
WINDOWS = [(0, 256), (64, 320), (192, 448), (256, 512)]


def _dwt_matrices(n):
    m = (n + _F - 1) // 2
    idx = np.concatenate([np.arange(_F - 2, -1, -1), np.arange(n),
                          np.arange(n - 1, n - _F, -1)])[1:]
    G_lo = np.zeros((m, n))
    G_hi = np.zeros((m, n))
    rev_lo = _DEC_LO[::-1]
    rev_hi = _DEC_HI[::-1]
    for i in range(m):
        for k in range(_F):
            t = 2 * i + k
            G_lo[i, idx[t]] += rev_lo[k]
            G_hi[i, idx[t]] += rev_hi[k]
    return G_lo, G_hi


def _idwt_matrices(n, m):
    up_len = 2 * m - 1
    S_lo = np.zeros((n, m))
    S_hi = np.zeros((n, m))
    for i in range(n):
        t = i + _F - 2
        for j_up in range(max(0, t - _F + 1), min(up_len, t + 1)):
            k = t - j_up
            if j_up % 2 == 0:
                S_lo[i, j_up // 2] += _REC_LO[k]
                S_hi[i, j_up // 2] += _REC_HI[k]
    return S_lo, S_hi


def _build_streams():
    """Returns s1 [4,128,256] (A^T windows), s2 [4,128,512] (B^T|C^T windows)."""
    _, G_hi = _dwt_matrices(N)
    _, S_hi = _idwt_matrices(N, M)
    E = S_hi @ G_hi
    A = np.eye(N) + E
    Bm = np.eye(N) - E
    Cm = 2.0 * E
    AT, BT, CT = A.T, Bm.T, Cm.T
    s1 = np.zeros((4, 128, 256), dtype=np.float32)
    s2 = np.zeros((4, 128, 512), dtype=np.float32)
    for rc, (lo, hi) in enumerate(WINDOWS):
        s1[rc] = AT[rc * 128:(rc + 1) * 128, lo:hi]
        s2[rc, :, :256] = BT[rc * 128:(rc + 1) * 128, lo:hi]
        s2[rc, :, 256:] = CT[rc * 128:(rc + 1) * 128, lo:hi]
    return s1, s2


# ---------------------------------------------------------------- bass build
_NC_CACHE = {}


def _build_nc(reps=1):
    import concourse.bacc as bacc
    import concourse.mybir as mybir
    from concourse.tile import TileContext

    F32 = mybir.dt.float32
    F32R = mybir.dt.float32r
    P = PLANES_PER_CORE

    nc = bacc.Bacc(None)
    data_d = nc.declare_dram_parameter("data", [P, N, N], F32R, isOutput=False)
    s1_d = nc.declare_dram_parameter("s1", [4, 128, 256], F32R, isOutput=False)
    s2_d = nc.declare_dram_parameter("s2", [4, 128, 512], F32R, isOutput=False)
    id_d = nc.declare_dram_parameter("ident", [128, 128], F32R, isOutput=False)
    out_d = nc.declare_dram_parameter("out", [P, N, N], F32, isOutput=True)

    with TileContext(nc) as tc:
        with (
            tc.tile_pool(name="const", bufs=1) as cpool,
            tc.tile_pool(name="xin", bufs=3) as xin,
            tc.tile_pool(name="mid", bufs=2) as mid,
            tc.tile_pool(name="oout", bufs=3) as oout,
            tc.tile_pool(name="ps", bufs=2, space="PSUM") as ps,
        ):
            s1_sb = cpool.tile([128, 4, 256], F32R)
            s2_sb = cpool.tile([128, 4, 512], F32R)
            id_sb = cpool.tile([128, 128], F32R)
            nc.sync.dma_start(out=s1_sb[:], in_=s1_d[:].rearrange("rc p w -> p rc w"))
            nc.sync.dma_start(out=s2_sb[:], in_=s2_d[:].rearrange("rc p w -> p rc w"))
            nc.sync.dma_start(out=id_sb[:], in_=id_d[:])

            for rep in range(reps):
                for plane in range(P):
                    x_sb = xin.tile([128, 4, N], F32R, tag="x")
                    nc.sync.dma_start(
                        out=x_sb[:],
                        in_=data_d[plane].rearrange("(rc p) c -> p rc c", p=128))

                    t1t_sb = mid.tile([128, 4, N], F32R, tag="t1t")
                    xt_sb = mid.tile([128, 4, N], F32R, tag="xt")
                    # ---- stage 1 ----
                    for wc in range(4):
                        ps_t = ps.tile([128, N], F32, tag="ps_t")
                        ps_x = ps.tile([128, N], F32R, tag="ps_x")
                        for rc in range(4):
                            lo, hi = WINDOWS[rc]
                            nc.tensor.matmul(
                                ps_t[:, lo:hi],
                                x_sb[:, rc, wc * 128:(wc + 1) * 128],
                                s1_sb[:, rc, :],
                                start=(rc == 0), stop=(rc == 3))
                        for rc in range(4):
                            nc.tensor.transpose(
                                ps_x[:, rc * 128:(rc + 1) * 128],
                                x_sb[:, rc, wc * 128:(wc + 1) * 128],
                                id_sb[:])
                        # ACT writing f32r faults the exec unit -> DVE only here
                        nc.vector.tensor_copy(t1t_sb[:, wc, :], ps_t[:])
                        nc.vector.tensor_copy(xt_sb[:, wc, :], ps_x[:])

                    # ---- stage 2 ----
                    o_sb = oout.tile([128, 4, N], F32, tag="o")
                    for ic in range(4):
                        ps_o = ps.tile([128, N], F32, tag="ps_o")
                        for kc in range(4):
                            lo, hi = WINDOWS[kc]
                            nc.tensor.matmul(
                                ps_o[:, lo:hi],
                                t1t_sb[:, kc, ic * 128:(ic + 1) * 128],
                                s2_sb[:, kc, 0:256],
                                start=(kc == 0), stop=False)
                            nc.tensor.matmul(
                                ps_o[:, lo:hi],
                                xt_sb[:, kc, ic * 128:(ic + 1) * 128],
                                s2_sb[:, kc, 256:512],
                                start=False, stop=(kc == 3))
                        nc.scalar.copy(o_sb[:, ic, :], ps_o[:])

                    nc.sync.dma_start(
                        out=out_d[plane].rearrange("(ic p) c -> p ic c", p=128),
                        in_=o_sb[:])

    nc.finalize()
    return nc


def _get_nc(reps=1):
    if reps not in _NC_CACHE:
        _NC_CACHE[reps] = _build_nc(reps)
    return _NC_CACHE[reps]


_STREAMS = None


def _get_streams():
    global _STREAMS
    if _STREAMS is None:
        _STREAMS = _build_streams()
    return _STREAMS


def _run(data96, reps=1):
    """data96: [96, 512, 512] f32. Returns [96, 512, 512] f32."""
    from concourse.bass_utils import run_bass_kernel_spmd

    s1, s2 = _get_streams()
    ident = np.eye(128, dtype=np.float32)
    nc = _get_nc(reps)
    in_maps = []
    for c in range(N_CORES):
        shard = np.ascontiguousarray(data96[c * PLANES_PER_CORE:(c + 1) * PLANES_PER_CORE])
        in_maps.append({"data": shard, "s1": s1, "s2": s2, "ident": ident})
    res = run_bass_kernel_spmd(nc, in_maps, list(range(N_CORES)))
    return np.concatenate([res.results[c]["out"] for c in range(N_CORES)], axis=0)


def kernel(data):
    data = np.asarray(data, dtype=np.float32)
    flat = data.reshape(B_TOT * C_TOT, N, N)
    out = _run(flat, reps=1)
    return out.reshape(B_TOT, C_TOT, N, N).astype(np.float32)


# revision 7
# speedup vs baseline: 3330.2316x; 3330.2316x over previous
"""Trainium2 Bass kernel for db4 wavelet high-frequency extraction.

Math: per (b,c) plane X [512,512]:
    out = A X B^T + X C^T
with A = I+E, B = I-E, C = 2E, E = S_hi @ G_hi (dwt/idwt high-band operator,
bandwidth 7).  This equals idwt2(ll, 2lh, 2hl, 2hh) of dwt2(X) (db4,
mode=symmetric) to ~2e-7 relative.

Mapping (per plane, all on the tensor engine, f32r = fp32 with 11-bit
mantissa rounding, full-rate on TRN2 when the moving operand is >=256 wide):
  stage 1: weights = X blocks (128x128), stream A^T band windows (N=256)
           -> PSUM accumulates T1^T = X^T A^T (banded windows cover [0,512))
           plus transpose-mode matmuls -> X^T
  stage 2: weights = T1^T / X^T blocks, stream B^T / C^T band windows
           -> out = T1 B^T + X C^T  in natural orientation.

Sharding: 96 (b,c) planes, 12 per core, pure data parallel on 8 cores.
"""
import numpy as np

# ---------------------------------------------------------------- constants
_DEC_LO = np.array([-0.010597401784997278, 0.032883011666982945,
                    0.030841381835986965, -0.18703481171888114,
                    -0.02798376941698385, 0.6308807679295904,
                    0.7148465705525415, 0.23037781330885523], dtype=np.float64)
_F = 8
_SIGNS = np.array([(-1.0) ** (k + 1) for k in range(_F)])
_DEC_HI = _SIGNS * _DEC_LO[::-1]
_REC_LO = _DEC_LO[::-1].copy()
_REC_HI = _DEC_HI[::-1].copy()

N = 512
M = (N + _F - 1) // 2
B_TOT, C_TOT, PLANES_PER_CORE, N_CORES = 32, 3, 12, 8
WINDOWS = [(0, 256), (64, 320), (192, 448), (256, 512)]


def _dwt_matrices(n):
    m = (n + _F - 1) // 2
    idx = np.concatenate([np.arange(_F - 2, -1, -1), np.arange(n),
                          np.arange(n - 1, n - _F, -1)])[1:]
    G_lo = np.zeros((m, n))
    G_hi = np.zeros((m, n))
    rev_lo = _DEC_LO[::-1]
    rev_hi = _DEC_HI[::-1]
    for i in range(m):
        for k in range(_F):
            t = 2 * i + k
            G_lo[i, idx[t]] += rev_lo[k]
            G_hi[i, idx[t]] += rev_hi[k]
    return G_lo, G_hi


def _idwt_matrices(n, m):
    up_len = 2 * m - 1
    S_lo = np.zeros((n, m))
    S_hi = np.zeros((n, m))
    for i in range(n):
        t = i + _F - 2
        for j_up in range(max(0, t - _F + 1), min(up_len, t + 1)):
            k = t - j_up
            if j_up % 2 == 0:
                S_lo[i, j_up // 2] += _REC_LO[k]
                S_hi[i, j_up // 2] += _REC_HI[k]
    return S_lo, S_hi


def _build_streams():
    """Returns s1 [4,128,256] (A^T windows), s2 [4,128,512] (B^T|C^T windows)."""
    _, G_hi = _dwt_matrices(N)
    _, S_hi = _idwt_matrices(N, M)
    E = S_hi @ G_hi
    A = np.eye(N) + E
    Bm = np.eye(N) - E
    Cm = 2.0 * E
    AT, BT, CT = A.T, Bm.T, Cm.T
    s1 = np.zeros((4, 128, 256), dtype=np.float32)
    s2 = np.zeros((4, 128, 512), dtype=np.float32)
    for rc, (lo, hi) in enumerate(WINDOWS):
        s1[rc] = AT[rc * 128:(rc + 1) * 128, lo:hi]
        s2[rc, :, :256] = BT[rc * 128:(rc + 1) * 128, lo:hi]
        s2[rc, :, 256:] = CT[rc * 128:(rc + 1) * 128, lo:hi]
    return s1, s2


# ---------------------------------------------------------------- bass build
_NC_CACHE = {}


def _build_nc(reps=1):
    import concourse.bacc as bacc
    import concourse.mybir as mybir
    from concourse.tile import TileContext

    F32 = mybir.dt.float32
    F32R = mybir.dt.float32r
    P = PLANES_PER_CORE

    nc = bacc.Bacc(None)
    data_d = nc.declare_dram_parameter("data", [P, N, N], F32R, isOutput=False)
    s1_d = nc.declare_dram_parameter("s1", [4, 128, 256], F32R, isOutput=False)
    s2_d = nc.declare_dram_parameter("s2", [4, 128, 512], F32R, isOutput=False)
    id_d = nc.declare_dram_parameter("ident", [128, 128], F32R, isOutput=False)
    out_d = nc.declare_dram_parameter("out", [P, N, N], F32, isOutput=True)

    with TileContext(nc) as tc:
        with (
            tc.tile_pool(name="const", bufs=1) as cpool,
            tc.tile_pool(name="xin", bufs=3) as xin,
            tc.tile_pool(name="mid", bufs=2) as mid,
            tc.tile_pool(name="oout", bufs=3) as oout,
            tc.tile_pool(name="ps", bufs=2, space="PSUM") as ps,
        ):
            s1_sb = cpool.tile([128, 4, 256], F32R)
            s2_sb = cpool.tile([128, 4, 512], F32R)
            id_sb = cpool.tile([128, 128], F32R)
            nc.sync.dma_start(out=s1_sb[:], in_=s1_d[:].rearrange("rc p w -> p rc w"))
            nc.sync.dma_start(out=s2_sb[:], in_=s2_d[:].rearrange("rc p w -> p rc w"))
            nc.sync.dma_start(out=id_sb[:], in_=id_d[:])

            for rep in range(reps):
                for plane in range(P):
                    x_sb = xin.tile([128, 4, N], F32R, tag="x")
                    nc.sync.dma_start(
                        out=x_sb[:],
                        in_=data_d[plane].rearrange("(rc p) c -> p rc c", p=128))

                    t1t_sb = mid.tile([128, 4, N], F32R, tag="t1t")
                    xt_sb = mid.tile([128, 4, N], F32R, tag="xt")
                    # ---- stage 1 ----
                    for wc in range(4):
                        ps_t = ps.tile([128, N], F32, tag="ps_t")
                        ps_x = ps.tile([128, N], F32R, tag="ps_x")
                        for rc in range(4):
                            lo, hi = WINDOWS[rc]
                            nc.tensor.matmul(
                                ps_t[:, lo:hi],
                                x_sb[:, rc, wc * 128:(wc + 1) * 128],
                                s1_sb[:, rc, :],
                                start=(rc == 0), stop=(rc == 3))
                        for rc in range(4):
                            nc.tensor.transpose(
                                ps_x[:, rc * 128:(rc + 1) * 128],
                                x_sb[:, rc, wc * 128:(wc + 1) * 128],
                                id_sb[:])
                        # ACT writing f32r faults the exec unit -> DVE only here
                        nc.vector.tensor_copy(t1t_sb[:, wc, :], ps_t[:])
                        nc.vector.tensor_copy(xt_sb[:, wc, :], ps_x[:])

                    # ---- stage 2 ----
                    o_sb = oout.tile([128, 4, N], F32, tag="o")
                    for ic in range(4):
                        ps_o = ps.tile([128, N], F32, tag="ps_o")
                        for kc in range(4):
                            lo, hi = WINDOWS[kc]
                            nc.tensor.matmul(
                                ps_o[:, lo:hi],
                                t1t_sb[:, kc, ic * 128:(ic + 1) * 128],
                                s2_sb[:, kc, 0:256],
                                start=(kc == 0), stop=False)
                            nc.tensor.matmul(
                                ps_o[:, lo:hi],
                                xt_sb[:, kc, ic * 128:(ic + 1) * 128],
                                s2_sb[:, kc, 256:512],
                                start=False, stop=(kc == 3))
                        nc.scalar.copy(o_sb[:, ic, :], ps_o[:])

                    nc.sync.dma_start(
                        out=out_d[plane].rearrange("(ic p) c -> p ic c", p=128),
                        in_=o_sb[:])

    nc.finalize()
    return nc


def _get_nc(reps=1):
    if reps not in _NC_CACHE:
        _NC_CACHE[reps] = _build_nc(reps)
    return _NC_CACHE[reps]


_STREAMS = None


def _get_streams():
    global _STREAMS
    if _STREAMS is None:
        _STREAMS = _build_streams()
    return _STREAMS


_RUNNERS = {}


def _make_runner(reps=1):
    """Build a persistent jitted SPMD callable for the kernel program.

    Mirrors concourse.bass2jax.run_bass_via_pjrt but caches the jitted
    function so repeated calls don't re-trace/re-hash the NEFF.
    """
    import jax
    import numpy as _np
    from jax.sharding import Mesh, PartitionSpec
    from jax.experimental.shard_map import shard_map
    import concourse.mybir as mybir
    from concourse import bass2jax

    bass2jax.install_neuronx_cc_hook()
    nc = _get_nc(reps)

    partition_name = (nc.partition_id_tensor.name
                      if nc.partition_id_tensor else None)
    in_names, out_names, out_avals, zero_outs = [], [], [], []
    for alloc in nc.m.functions[0].allocations:
        if not isinstance(alloc, mybir.MemoryLocationSet):
            continue
        name = alloc.memorylocations[0].name
        if alloc.kind == "ExternalInput":
            if name != partition_name:
                in_names.append(name)
        elif alloc.kind == "ExternalOutput":
            out_names.append(name)
            shape = tuple(alloc.tensor_shape)
            dtype = mybir.dt.np(alloc.dtype)
            out_avals.append(jax.core.ShapedArray(shape, dtype))
            zero_outs.append(_np.zeros(shape, dtype))
    n_params = len(in_names)
    n_outs = len(out_avals)
    all_in_names = in_names + out_names
    if partition_name is not None:
        all_in_names.append(partition_name)
    donate = tuple(range(n_params, n_params + n_outs))

    def _body(*args):
        operands = list(args)
        if partition_name is not None:
            operands.append(bass2jax.partition_id_tensor())
        outs = bass2jax._bass_exec_p.bind(
            *operands,
            out_avals=tuple(out_avals),
            in_names=tuple(all_in_names),
            out_names=tuple(out_names),
            lowering_input_output_aliases=(),
            sim_require_finite=True,
            sim_require_nnan=True,
            nc=nc,
        )
        return tuple(outs)

    devices = jax.devices()[:N_CORES]
    mesh = Mesh(np.asarray(devices), ("core",))
    in_specs = (PartitionSpec("core"),) * (n_params + n_outs)
    out_specs = (PartitionSpec("core"),) * n_outs
    sharded = jax.jit(
        shard_map(_body, mesh=mesh, in_specs=in_specs, out_specs=out_specs,
                  check_rep=False),
        donate_argnums=donate, keep_unused=True)

    def _concat_in(per_core_inputs):
        return [
            _np.concatenate([_np.asarray(per_core_inputs[c][nm])
                             for c in range(N_CORES)], axis=0)
            for nm in in_names
        ]

    def run(per_core_inputs):
        """per_core_inputs: list over cores of dict name->np array."""
        concat_zeros = [
            _np.zeros((N_CORES * z.shape[0], *z.shape[1:]), z.dtype)
            for z in zero_outs
        ]
        out_arrs = sharded(*_concat_in(per_core_inputs), *concat_zeros)
        jax.block_until_ready(out_arrs)
        return {
            nm: _np.asarray(out_arrs[i]).reshape(N_CORES, *out_avals[i].shape)
            for i, nm in enumerate(out_names)
        }

    def timeit(per_core_inputs, iters=10, warmup=3):
        """Device-resident timing: returns list of per-call wall seconds."""
        import time as _time
        import jax.numpy as jnp
        from jax.sharding import NamedSharding

        shd = NamedSharding(mesh, PartitionSpec("core"))
        dev_in = [jax.device_put(a, shd) for a in _concat_in(per_core_inputs)]
        zero_shapes = [(N_CORES * z.shape[0], *z.shape[1:]) for z in zero_outs]
        zeros_fn = jax.jit(
            lambda: tuple(jnp.zeros(s, z.dtype)
                          for s, z in zip(zero_shapes, zero_outs)),
            out_shardings=tuple(shd for _ in zero_outs))
        times = []
        for i in range(warmup + iters):
            zs = jax.block_until_ready(zeros_fn())
            t0 = _time.perf_counter()
            out_arrs = sharded(*dev_in, *zs)
            jax.block_until_ready(out_arrs)
            dt = _time.perf_counter() - t0
            if i >= warmup:
                times.append(dt)
        return times

    run.timeit = timeit
    return run


def _get_runner(reps=1):
    if reps not in _RUNNERS:
        _RUNNERS[reps] = _make_runner(reps)
    return _RUNNERS[reps]


def _in_maps(data96):
    s1, s2 = _get_streams()
    ident = np.eye(128, dtype=np.float32)
    return [
        {"data": np.ascontiguousarray(
            data96[c * PLANES_PER_CORE:(c + 1) * PLANES_PER_CORE]),
         "s1": s1, "s2": s2, "ident": ident}
        for c in range(N_CORES)
    ]


def _run(data96, reps=1):
    """data96: [96, 512, 512] f32. Returns [96, 512, 512] f32."""
    run = _get_runner(reps)
    outs = run(_in_maps(data96))
    return outs["out"].reshape(96, N, N)


def kernel(data):
    data = np.asarray(data, dtype=np.float32)
    flat = data.reshape(B_TOT * C_TOT, N, N)
    out = _run(flat, reps=1)
    return out.reshape(B_TOT, C_TOT, N, N).astype(np.float32)
